# revision 38
# baseline (speedup 1.0000x reference)
"""Tensor-parallel causal attention block for 8 Trainium2 NeuronCores.

Sharding: heads split across cores (2 of 16 heads per core).  Each core
computes q/k/v projections for its head slice (columns of wq/wk/wv), RoPE,
causal attention, and a row-slice of the output projection (rows of wo),
producing a partial full-shape output; the host sums the 8 partials.

All matmuls run in bf16 (1 cycle/row on the PE, same as fp32r, but half the
LDWEIGHTS time and half the DMA bytes).  Scores are computed transposed
(S^T[k, q]) so the softmax renormalization folds into PE ones-matmuls and P
needs no transposes before P@V.  The RoPE rotate-half partition swap runs as
two SBUF->SBUF copies issued from the ACT queue (the sync queue carries only
input/output DMAs, issued few-and-large because each issue costs ~600ns of
queue time regardless of size; x quarters lead, constants trail).  V is
projected transposed and moved to natural layout by PE transposes into a
single shared PSUM bank.  Diagonal causal blocks are q-sliced so
fully-masked regions are never computed.  Projection PSUM tiles drain under
the NEXT projection block's matmuls so the PE never waits; phases A/B/C stay
separate on purpose — merging them oversubscribes SBUF/PSUM ports and slows
every matmul by ~20%.  Output partials are written in bf16 and summed on the
host in fp32.
"""

import math
import sys

sys.path.insert(0, "/opt/trn_rl_repo")

import numpy as np
import ml_dtypes

B = 2
S = 2048
E = 2048
H = 16
D = 128
ROPE_BASE = 10000.0
NCORES = 8
HPC = H // NCORES          # heads per core = 2
DC = HPC * D               # head-dim cols per core = 256
T = B * S                  # 4096 tokens
KC = E // 128              # 16 contraction chunks
TC8 = 512                  # token chunk for projections
NTC8 = S // TC8            # 4 per batch
SB512 = 512                # query super-block
NSB = S // SB512           # 4 per batch
SCALE = 1.0 / math.sqrt(D)
_COMPILED = None


def _build_program():
    import concourse.bass as bass
    import concourse.mybir as mybir
    from concourse import bacc
    from concourse.tile import TileContext

    f32 = mybir.dt.float32
    bf16 = mybir.dt.bfloat16

    def fr(ap):
        return ap.bitcast(mybir.dt.float32r)

    nc = bacc.Bacc()
    # host-blocked layouts: every DMA moves a contiguous per-partition span,
    # and DMA COUNT is minimized (each sync-queue dma_start costs ~600ns of
    # queue occupancy regardless of size)
    xT_d = nc.declare_dram_parameter("xT", [B * NTC8, 128, KC, TC8], bf16, isOutput=False)
    cos_d = nc.declare_dram_parameter("cosF", [128, S], bf16, isOutput=False)
    sin_d = nc.declare_dram_parameter("sinF", [128, S], bf16, isOutput=False)
    wq_d = nc.declare_dram_parameter("wq", [128, KC, DC], bf16, isOutput=False)
    wk_d = nc.declare_dram_parameter("wk", [128, KC, DC], bf16, isOutput=False)
    wv_d = nc.declare_dram_parameter("wv", [128, KC, DC], bf16, isOutput=False)
    wo_d = nc.declare_dram_parameter("wo", [128, HPC, E], bf16, isOutput=False)
    on_d = nc.declare_dram_parameter("ones", [128, 128], bf16, isOutput=False)
    id_d = nc.declare_dram_parameter("ident", [128, 128], f32, isOutput=False)
    out_d = nc.declare_dram_parameter("out", [B * (S // 128), 128, E], bf16, isOutput=True)

    Exp = mybir.ActivationFunctionType.Exp
    mult = mybir.AluOpType.mult
    add = mybir.AluOpType.add

    with TileContext(nc) as tc:
        with (
            tc.tile_pool(name="wpool", bufs=1) as wp,
            tc.tile_pool(name="persist", bufs=1) as pp,
            tc.tile_pool(name="xin", bufs=16) as xp,
            tc.tile_pool(name="rope", bufs=3) as rp,
            tc.tile_pool(name="ptile", bufs=6) as ptp,
            tc.tile_pool(name="small", bufs=2) as smp,
            tc.tile_pool(name="outsb", bufs=4) as op,
        ):
            # ---- resident weights / constants ----
            wq_sb = wp.tile([128, KC, DC], bf16)
            wk_sb = wp.tile([128, KC, DC], bf16)
            wv_sb = wp.tile([128, KC, DC], bf16)
            wo_sb = wp.tile([128, HPC, E], bf16)
            cos_sb = wp.tile([128, S], bf16)
            sin_sb = wp.tile([128, S], bf16)
            ones_sb = wp.tile([128, 128], bf16)
            ident_sb = wp.tile([128, 128], f32)

            # ---- persistent per-batch arrays (slots reused across batches) ----
            # per-512-block tiles (not one [128, S] tile) so a reader of
            # block i never waits on a writer of block j != i
            qT = [[pp.tile([128, TC8], bf16, name=f"qT{h}_{c}", tag=f"qT{h}_{c}")
                   for c in range(NTC8)] for h in range(HPC)]
            kT = [[pp.tile([128, TC8], bf16, name=f"kT{h}_{c}", tag=f"kT{h}_{c}")
                   for c in range(NTC8)] for h in range(HPC)]
            v_sb = pp.tile([128, S // 128, DC], bf16, name="v_sb", tag="v")
            zn = [[pp.tile([128, SB512], bf16, name=f"zn{h}_{c}", tag=f"zn{h}_{c}")
                   for c in range(NSB)] for h in range(HPC)]

            # x DMAs for a whole batch, emitted up-front (b=0 at phase-A
            # start, b=1 during b=0's phase B) so the in-order sync queue
            # never delays an x load behind dependent work
            xq_tiles = {}

            def emit_x_dmas(bb, with_weights=False):
                # issue order matters: the sync queue serializes issues and the
                # DMA engines drain them roughly in order, so x quarters lead
                # (the PE consumes them immediately) and weights interleave
                # just-in-time; constants trail behind everything
                tiles = []
                for tc8 in range(NTC8):
                    for qtr in range(4):
                        xq = xp.tile([128, 4, TC8], bf16, name="xq", tag="xq")
                        if with_weights and tc8 == 0 and qtr == 0:
                            # first quarter in two pieces so the very first
                            # matmul starts after a 64KB transfer
                            nc.sync.dma_start(out=xq[:, 0:1, :], in_=xT_d[0, :, 0:1, :])
                            nc.sync.dma_start(out=wq_sb[:, 0:4, :], in_=wq_d[:, 0:4, :])
                            nc.sync.dma_start(out=xq[:, 1:4, :], in_=xT_d[0, :, 1:4, :])
                            nc.sync.dma_start(out=wq_sb[:, 4:KC, :], in_=wq_d[:, 4:KC, :])
                        else:
                            nc.sync.dma_start(
                                out=xq[:], in_=xT_d[bb * NTC8 + tc8, :, qtr * 4:(qtr + 1) * 4, :])
                        tiles.append(xq)
                    if with_weights and tc8 == 0:
                        nc.sync.dma_start(out=wk_sb[:], in_=wk_d[:])
                    if with_weights and tc8 == 1:
                        nc.sync.dma_start(out=wv_sb[:], in_=wv_d[:])
                xq_tiles[bb] = tiles

            warm_sb = wp.tile([128, 512], bf16)

            for b in range(B):
                # ============ Phase A: projections + RoPE + V transpose ============
                with (
                    tc.tile_pool(name=f"psA{b}", bufs=1, space="PSUM") as pA,
                    tc.tile_pool(name=f"psR{b}", bufs=2, space="PSUM") as pR,
                ):
                    if b == 0:
                        nc.vector.memset(warm_sb[:], 0.0)
                        warm_ps = pA.tile([128, TC8], f32, name="warm", tag="pq0")
                        for _ in range(8):
                            nc.tensor.matmul(warm_ps[:], lhsT=warm_sb[:, 0:128], rhs=warm_sb[:],
                                             start=True, stop=True)
                        emit_x_dmas(0, with_weights=True)
                        nc.sync.dma_start(out=cos_sb[:], in_=cos_d[:])
                        nc.sync.dma_start(out=sin_sb[:], in_=sin_d[:])
                        nc.sync.dma_start(out=fr(ident_sb[:]), in_=fr(id_d[:]))
                        nc.sync.dma_start(out=ones_sb[:], in_=on_d[:])
                        nc.sync.dma_start(out=wo_sb[:], in_=wo_d[:])
                    for tc8 in range(NTC8):
                        s0 = tc8 * TC8
                        xqs = xq_tiles[b][tc8 * 4:(tc8 + 1) * 4]

                        def xts(kc):
                            return xqs[kc // 4][:, kc % 4, :]

                        q_ps = [pA.tile([128, TC8], f32, name=f"q_ps{h}", tag=f"pq{h}") for h in range(HPC)]
                        k_ps = [pA.tile([128, TC8], f32, name=f"k_ps{h}", tag=f"pk{h}") for h in range(HPC)]
                        v_ps = [pA.tile([128, TC8], f32, name=f"v_ps{h}", tag=f"pv{h}") for h in range(HPC)]

                        # q/k projection blocks: one PSUM tile per block; the
                        # drain (ACT copy + rot swap + DVE RoPE) for block i
                        # is emitted under block i+1's matmuls so the PE
                        # never waits on a drain.
                        qk_seq = [(q_ps[0], wq_sb, 0, qT[0][tc8]), (q_ps[1], wq_sb, 1, qT[1][tc8]),
                                  (k_ps[0], wk_sb, 0, kT[0][tc8]), (k_ps[1], wk_sb, 1, kT[1][tc8])]
                        pending = []   # (tmp, dst) waiting for swap+DVE emission

                        def emit_rope_tail(tmp, dst):
                            # dst is the per-chunk [128, TC8] tile for this block
                            # rotate-half via two SBUF->SBUF half-swap copies
                            # issued from the ACT queue (the PE and sync queue
                            # stay free; ~0.7us each in bf16)
                            rot = rp.tile([128, TC8], bf16, name="rot", tag="rot")
                            nc.scalar.dma_start(out=rot[0:64, :], in_=tmp[64:128, :])
                            nc.scalar.dma_start(out=rot[64:128, :], in_=tmp[0:64, :])
                            nc.vector.tensor_tensor(tmp[:], tmp[:], cos_sb[:, s0:s0 + TC8], mult)
                            nc.vector.tensor_tensor(rot[:], rot[:], sin_sb[:, s0:s0 + TC8], mult)
                            nc.vector.tensor_tensor(dst[:], tmp[:], rot[:], add)

                        for ps, w_sb, h, dst in qk_seq:
                            for kc in range(KC):
                                nc.tensor.matmul(ps[:], lhsT=w_sb[:, kc, h * D:(h + 1) * D],
                                                 rhs=xts(kc), start=(kc == 0), stop=(kc == KC - 1))
                            if pending:
                                emit_rope_tail(*pending.pop())
                            tmp = rp.tile([128, TC8], bf16, name="tmp", tag="tmp")
                            nc.scalar.copy(tmp[:], ps[:])
                            pending.append((tmp, dst))

                        # v blocks: transposed projection, drained via ACT copy
                        # + XBAR DMA-transpose into natural layout.
                        for h in range(HPC):
                            for kc in range(KC):
                                nc.tensor.matmul(v_ps[h][:], lhsT=wv_sb[:, kc, h * D:(h + 1) * D],
                                                 rhs=xts(kc), start=(kc == 0), stop=(kc == KC - 1))
                            if pending:
                                emit_rope_tail(*pending.pop())
                            vt = rp.tile([128, TC8], f32, name="vt", tag="vt", bufs=2)
                            nc.scalar.copy(fr(vt[:]), v_ps[h][:])
                            # PE transposes to natural layout (fp32r, 1.5
                            # cyc/row, ~130ns each); the [128,4,128] f32 tp
                            # tile is byte-identical to a rot_ps slot so it
                            # shares the same 2 PSUM banks
                            tp = pR.tile([128, 4, 128], f32, name="tp", tag="tp")
                            for tb in range(TC8 // 128):
                                nc.tensor.matmul(fr(tp[:, tb, :]), lhsT=fr(vt[:, tb * 128:(tb + 1) * 128]),
                                                 rhs=fr(ident_sb[:]), is_transpose=True,
                                                 skip_group_check=True)
                            nc.vector.tensor_copy(v_sb[:, s0 // 128:s0 // 128 + 4, h * D:(h + 1) * D],
                                                  tp[:, 0:4, :])

                # ============ Phase B: causal attention ============
                with tc.tile_pool(name=f"psB{b}", bufs=1, space="PSUM") as pB:
                    if b + 1 < B:
                        emit_x_dmas(b + 1)
                    for sb in range(NSB):
                        for h in range(HPC):
                            nkb = (sb + 1) * (SB512 // 128)
                            pts = [None] * nkb

                            def emit_score(kblk):
                                delta = kblk - sb * (SB512 // 128)
                                q0 = 128 * delta if delta > 0 else 0
                                W = SB512 - q0
                                q_sl = qT[h][sb][:, q0:SB512]
                                st_ps = pB.tile([128, SB512], f32, name="st_ps", tag="st", bufs=4)
                                kt_sl = kT[h][kblk // 4][:, (kblk % 4) * 128:(kblk % 4 + 1) * 128]
                                nc.tensor.matmul(st_ps[:, :W], lhsT=kt_sl,
                                                 rhs=q_sl, start=True, stop=True)
                                pt = ptp.tile([128, SB512], bf16, name="pt", tag="pt")
                                nc.scalar.activation(pt[:, :W], st_ps[:, :W], Exp, scale=SCALE)
                                if delta >= 0:
                                    nc.gpsimd.affine_select(
                                        out=pt[:, :W], in_=pt[:, :W],
                                        pattern=[[1, W]], compare_op=mybir.AluOpType.is_ge,
                                        fill=0.0, base=0, channel_multiplier=-1,
                                    )
                                pts[kblk] = (pt, q0, W)

                            def emit_zsum(kblk):
                                pt, q0, W = pts[kblk]
                                nc.tensor.matmul(z_ps[:, q0:SB512], lhsT=v_sb[:, kblk, h * D:(h + 1) * D],
                                                 rhs=pt[:, :W], start=(kblk == 0), stop=(kblk == nkb - 1))
                                nc.tensor.matmul(sum_ps[:, q0:SB512], lhsT=ones_sb[:],
                                                 rhs=pt[:, :W], start=(kblk == 0), stop=(kblk == nkb - 1))
                                pts[kblk] = None

                            for kblk in range(min(4, nkb)):
                                emit_score(kblk)
                            z_ps = pB.tile([128, SB512], f32, name="z_ps", tag="z", bufs=2)
                            sum_ps = pB.tile([128, SB512], f32, name="sum_ps", tag="sum", bufs=2)
                            for kblk in range(nkb):
                                if kblk + 4 < nkb:
                                    emit_score(kblk + 4)
                                emit_zsum(kblk)
                            rep_sb = smp.tile([128, SB512], f32, name="rep_sb", tag="repsb")
                            nc.vector.reciprocal_approx_fast(out=rep_sb[:], in_=sum_ps[:])
                            nc.vector.tensor_tensor(zn[h][sb][:], z_ps[:], rep_sb[:], mult)

                # ============ Phase C: output projection ============
                with tc.tile_pool(name=f"psC{b}", bufs=4, space="PSUM") as pC:
                    for tb in range(S // 128):
                        o_sb = op.tile([128, E], bf16, name="o_sb", tag="osb")
                        for ec in range(E // 512):
                            o_ps = pC.tile([128, 512], f32, name="o_ps", tag="o")
                            for h in range(HPC):
                                zn_sl = zn[h][tb // 4][:, (tb % 4) * 128:(tb % 4 + 1) * 128]
                                nc.tensor.matmul(o_ps[:], lhsT=zn_sl,
                                                 rhs=wo_sb[:, h, ec * 512:(ec + 1) * 512],
                                                 start=(h == 0), stop=(h == HPC - 1))
                            if ec % 2 == 0:
                                nc.vector.tensor_copy(o_sb[:, ec * 512:(ec + 1) * 512], o_ps[:])
                            else:
                                nc.scalar.copy(o_sb[:, ec * 512:(ec + 1) * 512], o_ps[:])
                        if b == B - 1 and tb >= S // 128 - 2:
                            # shorten the final drain: last tiles ship in halves
                            nc.sync.dma_start(out=out_d[b * (S // 128) + tb, :, 0:E // 2],
                                              in_=o_sb[:, 0:E // 2])
                            nc.sync.dma_start(out=out_d[b * (S // 128) + tb, :, E // 2:E],
                                              in_=o_sb[:, E // 2:E])
                        else:
                            nc.sync.dma_start(out=out_d[b * (S // 128) + tb], in_=o_sb[:])

    nc.compile()
    return nc


def _get_compiled():
    global _COMPILED
    if _COMPILED is None:
        _COMPILED = _build_program()
    return _COMPILED


def _host_inputs(x, wq, wk, wv, wo):
    bf = ml_dtypes.bfloat16
    x = np.asarray(x, dtype=np.float32)
    # xT blocked: [B*NTC8, 128, KC, TC8]; element (b*NTC8+tc8, p, kc, c) = x[b, tc8*TC8+c, kc*128+p]
    xT = np.ascontiguousarray(
        x.reshape(B, NTC8, TC8, KC, 128).transpose(0, 1, 4, 3, 2).reshape(B * NTC8, 128, KC, TC8)
    ).astype(bf)

    pos = np.arange(S, dtype=np.float32)
    inv_freq = (1.0 / (ROPE_BASE ** (np.arange(0, D, 2, dtype=np.float32) / np.float32(D)))).astype(np.float32)
    ang = pos[:, None] * inv_freq[None, :]          # (S, 64) fp32
    cos_h = np.cos(ang).astype(np.float32)
    sin_h = np.sin(ang).astype(np.float32)
    cosF = np.ascontiguousarray(np.concatenate([cos_h.T, cos_h.T], axis=0)).astype(bf)   # (128, S)
    sinF = np.ascontiguousarray(np.concatenate([-sin_h.T, sin_h.T], axis=0)).astype(bf)  # (128, S)
    ones = np.ones((128, 128), dtype=np.float32).astype(bf)
    ident = np.eye(128, dtype=np.float32)

    wq = np.asarray(wq, dtype=np.float32)
    wk = np.asarray(wk, dtype=np.float32)
    wv = np.asarray(wv, dtype=np.float32)
    wo = np.asarray(wo, dtype=np.float32)

    maps = []
    for c in range(NCORES):
        sl = slice(c * DC, (c + 1) * DC)
        maps.append({
            "xT": xT,
            "cosF": cosF,
            "sinF": sinF,
            "wq": np.ascontiguousarray(wq[:, sl].reshape(KC, 128, DC).transpose(1, 0, 2)).astype(bf),
            "wk": np.ascontiguousarray(wk[:, sl].reshape(KC, 128, DC).transpose(1, 0, 2)).astype(bf),
            "wv": np.ascontiguousarray(wv[:, sl].reshape(KC, 128, DC).transpose(1, 0, 2)).astype(bf),
            "wo": np.ascontiguousarray(wo[sl, :].reshape(HPC, 128, E).transpose(1, 0, 2)).astype(bf),
            "ones": ones,
            "ident": ident,
        })
    return maps


def kernel(x, wq, wk, wv, wo, _trace=False):
    from concourse.bass_utils import run_bass_kernel_spmd

    nc = _get_compiled()
    maps = _host_inputs(x, wq, wk, wv, wo)
    res = run_bass_kernel_spmd(nc, maps, list(range(NCORES)), trace=_trace)
    total = np.zeros((B * (S // 128), 128, E), dtype=np.float32)
    for c in range(NCORES):
        total += res.results[c]["out"].astype(np.float32)
    out = total.reshape(B, S, E)
    if _trace:
        kernel.last_exec_time_ns = res.exec_time_ns
        kernel.last_trace = res.instructions_and_trace
    return out


# revision 39
# speedup vs baseline: 1.1787x; 1.1787x over previous
"""Tensor-parallel causal attention block for 8 Trainium2 NeuronCores.

Sharding: heads split across cores (2 of 16 heads per core).  Each core
computes q/k/v projections for its head slice (columns of wq/wk/wv), RoPE,
causal attention, and a row-slice of the output projection (rows of wo),
producing a partial full-shape output; the host sums the 8 partials.

All matmuls run in bf16 (1 cycle/row on the PE, same as fp32r, but half the
LDWEIGHTS time and half the DMA bytes).  Scores are computed transposed
(S^T[k, q]) so the softmax renormalization folds into PE ones-matmuls and P
needs no transposes before P@V.  The RoPE rotate-half partition swap runs as
a PE permutation matmul (keeps the sync queue free for real DMAs).  V is
projected transposed and moved to natural layout via XBAR DMA-transposes.
Diagonal causal blocks are q-sliced so fully-masked regions are never
computed.  All DRAM I/O uses host-blocked layouts so every DMA moves one
contiguous tile; the output partials are written in bf16.
"""

import math
import sys

sys.path.insert(0, "/opt/trn_rl_repo")

import numpy as np
import ml_dtypes

B = 2
S = 2048
E = 2048
H = 16
D = 128
ROPE_BASE = 10000.0
NCORES = 8
HPC = H // NCORES          # heads per core = 2
DC = HPC * D               # head-dim cols per core = 256
T = B * S                  # 4096 tokens
KC = E // 128              # 16 contraction chunks
TC8 = 512                  # token chunk for projections
NTC8 = S // TC8            # 4 per batch
SB512 = 512                # query super-block
NSB = S // SB512           # 4 per batch
SCALE = 1.0 / math.sqrt(D)
LOOKAHEAD = 3              # score matmuls emitted ahead of z/sum matmuls

_COMPILED = None


def _build_program():
    import concourse.bass as bass
    import concourse.mybir as mybir
    from concourse import bacc
    from concourse.tile import TileContext

    f32 = mybir.dt.float32
    bf16 = mybir.dt.bfloat16

    def fr(ap):
        return ap.bitcast(mybir.dt.float32r)

    nc = bacc.Bacc()
    # host-blocked layouts: every DMA moves a contiguous per-partition span,
    # and DMA COUNT is minimized (each sync-queue dma_start costs ~600ns of
    # queue occupancy regardless of size)
    xT_d = nc.declare_dram_parameter("xT", [B * NTC8, 128, KC, TC8], bf16, isOutput=False)
    cos_d = nc.declare_dram_parameter("cosF", [128, S], bf16, isOutput=False)
    sin_d = nc.declare_dram_parameter("sinF", [128, S], bf16, isOutput=False)
    wq_d = nc.declare_dram_parameter("wq", [128, KC, DC], bf16, isOutput=False)
    wk_d = nc.declare_dram_parameter("wk", [128, KC, DC], bf16, isOutput=False)
    wv_d = nc.declare_dram_parameter("wv", [128, KC, DC], bf16, isOutput=False)
    wo_d = nc.declare_dram_parameter("wo", [128, HPC, E], bf16, isOutput=False)
    on_d = nc.declare_dram_parameter("ones", [128, 128], bf16, isOutput=False)
    id_d = nc.declare_dram_parameter("ident", [128, 128], f32, isOutput=False)
    out_d = nc.declare_dram_parameter("out", [B * (S // 128), 128, E], bf16, isOutput=True)

    Exp = mybir.ActivationFunctionType.Exp
    mult = mybir.AluOpType.mult
    add = mybir.AluOpType.add

    with TileContext(nc) as tc:
        with (
            tc.tile_pool(name="wpool", bufs=1) as wp,
            tc.tile_pool(name="persist", bufs=1) as pp,
            tc.tile_pool(name="xin", bufs=16) as xp,
            tc.tile_pool(name="rope", bufs=3) as rp,
            tc.tile_pool(name="ptile", bufs=6) as ptp,
            tc.tile_pool(name="small", bufs=2) as smp,
            tc.tile_pool(name="outsb", bufs=4) as op,
        ):
            # ---- resident weights / constants (DMAs deferred: the qkv weight
            # chunks stream inside the first kc loop so the first xt tile is
            # not queued behind the constants on the sync ring) ----
            wq_sb = wp.tile([128, KC, DC], bf16)
            wk_sb = wp.tile([128, KC, DC], bf16)
            wv_sb = wp.tile([128, KC, DC], bf16)
            wo_sb = wp.tile([128, HPC, E], bf16)
            cos_sb = wp.tile([128, S], bf16)
            sin_sb = wp.tile([128, S], bf16)
            ones_sb = wp.tile([128, 128], bf16)
            ident_sb = wp.tile([128, 128], f32)

            # ---- persistent per-batch arrays (slots reused across batches) ----
            qT = [pp.tile([128, S], bf16, name=f"qT{h}", tag=f"qT{h}") for h in range(HPC)]
            kT = [pp.tile([128, S], bf16, name=f"kT{h}", tag=f"kT{h}") for h in range(HPC)]
            v_sb = pp.tile([128, S // 128, DC], bf16, name="v_sb", tag="v")
            zn = [pp.tile([128, S], bf16, name=f"zn{h}", tag=f"zn{h}") for h in range(HPC)]

            # x DMAs for a whole batch, emitted up-front (b=0 at phase-A
            # start, b=1 during b=0's phase B) so the in-order sync queue
            # never delays an x load behind dependent work
            xq_tiles = {}

            def emit_x_dmas(bb, with_weights=False):
                # issue order matters: the sync queue serializes issues and the
                # DMA engines drain them roughly in order, so x quarters lead
                # (the PE consumes them immediately) and weights interleave
                # just-in-time; constants trail behind everything
                tiles = []
                for tc8 in range(NTC8):
                    for qtr in range(4):
                        xq = xp.tile([128, 4, TC8], bf16, name="xq", tag="xq")
                        if with_weights and tc8 == 0 and qtr == 0:
                            # first quarter in two pieces so the very first
                            # matmul starts after a 64KB transfer
                            nc.sync.dma_start(out=xq[:, 0:1, :], in_=xT_d[0, :, 0:1, :])
                            nc.sync.dma_start(out=wq_sb[:, 0:4, :], in_=wq_d[:, 0:4, :])
                            nc.sync.dma_start(out=xq[:, 1:4, :], in_=xT_d[0, :, 1:4, :])
                            nc.sync.dma_start(out=wq_sb[:, 4:KC, :], in_=wq_d[:, 4:KC, :])
                        else:
                            nc.sync.dma_start(
                                out=xq[:], in_=xT_d[bb * NTC8 + tc8, :, qtr * 4:(qtr + 1) * 4, :])
                        tiles.append(xq)
                    if with_weights and tc8 == 0:
                        nc.sync.dma_start(out=wk_sb[:], in_=wk_d[:])
                    if with_weights and tc8 == 1:
                        nc.sync.dma_start(out=wv_sb[:], in_=wv_d[:])
                xq_tiles[bb] = tiles

            warm_sb = wp.tile([128, 512], bf16)

            for b in range(B):
                # ============ Phase A: projections + RoPE + V transpose ============
                with (
                    tc.tile_pool(name=f"psA{b}", bufs=1, space="PSUM") as pA,
                    tc.tile_pool(name=f"psR{b}", bufs=2, space="PSUM") as pR,
                ):
                    if b == 0:
                        # PE warm-up: dummy matmuls on a zeroed tile ramp the
                        # clock during the initial DMA wait; the first real
                        # start=True projection matmul overwrites the slot
                        nc.vector.memset(warm_sb[:], 0.0)
                        warm_ps = pA.tile([128, TC8], f32, name="warm", tag="pq0")
                        for _ in range(8):
                            nc.tensor.matmul(warm_ps[:], lhsT=warm_sb[:, 0:128], rhs=warm_sb[:],
                                             start=True, stop=True)
                        emit_x_dmas(0, with_weights=True)
                        nc.sync.dma_start(out=cos_sb[:], in_=cos_d[:])
                        nc.sync.dma_start(out=sin_sb[:], in_=sin_d[:])
                        nc.sync.dma_start(out=fr(ident_sb[:]), in_=fr(id_d[:]))
                        nc.sync.dma_start(out=ones_sb[:], in_=on_d[:])
                        nc.sync.dma_start(out=wo_sb[:], in_=wo_d[:])
                    for tc8 in range(NTC8):
                        s0 = tc8 * TC8
                        xqs = xq_tiles[b][tc8 * 4:(tc8 + 1) * 4]

                        def xts(kc):
                            return xqs[kc // 4][:, kc % 4, :]

                        q_ps = [pA.tile([128, TC8], f32, name=f"q_ps{h}", tag=f"pq{h}") for h in range(HPC)]
                        k_ps = [pA.tile([128, TC8], f32, name=f"k_ps{h}", tag=f"pk{h}") for h in range(HPC)]
                        v_ps = [pA.tile([128, TC8], f32, name=f"v_ps{h}", tag=f"pv{h}") for h in range(HPC)]

                        # q/k projection blocks: one PSUM tile per block; the
                        # drain (ACT copy + PE swap-matmul + DVE RoPE) for
                        # block i is emitted under block i+1's matmuls so the
                        # PE never waits on a drain.
                        qk_seq = [(q_ps[0], wq_sb, 0, qT[0]), (q_ps[1], wq_sb, 1, qT[1]),
                                  (k_ps[0], wk_sb, 0, kT[0]), (k_ps[1], wk_sb, 1, kT[1])]
                        pending = []   # (tmp, dst) waiting for swap+DVE emission

                        def emit_rope_tail(tmp, dst):
                            # rotate-half via two SBUF->SBUF half-swap copies
                            # issued from the ACT queue (the PE and sync queue
                            # stay free; ~0.7us each in bf16)
                            rot = rp.tile([128, TC8], bf16, name="rot", tag="rot")
                            nc.scalar.dma_start(out=rot[0:64, :], in_=tmp[64:128, :])
                            nc.scalar.dma_start(out=rot[64:128, :], in_=tmp[0:64, :])
                            nc.vector.tensor_tensor(tmp[:], tmp[:], cos_sb[:, s0:s0 + TC8], mult)
                            nc.vector.tensor_tensor(rot[:], rot[:], sin_sb[:, s0:s0 + TC8], mult)
                            nc.vector.tensor_tensor(dst[:, s0:s0 + TC8], tmp[:], rot[:], add)

                        for ps, w_sb, h, dst in qk_seq:
                            for kc in range(KC):
                                nc.tensor.matmul(ps[:], lhsT=w_sb[:, kc, h * D:(h + 1) * D],
                                                 rhs=xts(kc), start=(kc == 0), stop=(kc == KC - 1))
                            if pending:
                                emit_rope_tail(*pending.pop())
                            tmp = rp.tile([128, TC8], bf16, name="tmp", tag="tmp")
                            nc.scalar.copy(tmp[:], ps[:])
                            pending.append((tmp, dst))

                        # v blocks: transposed projection, drained via ACT copy
                        # + XBAR DMA-transpose into natural layout.
                        for h in range(HPC):
                            for kc in range(KC):
                                nc.tensor.matmul(v_ps[h][:], lhsT=wv_sb[:, kc, h * D:(h + 1) * D],
                                                 rhs=xts(kc), start=(kc == 0), stop=(kc == KC - 1))
                            if pending:
                                emit_rope_tail(*pending.pop())
                            vt = rp.tile([128, TC8], f32, name="vt", tag="vt", bufs=2)
                            nc.scalar.copy(fr(vt[:]), v_ps[h][:])
                            # PE transposes to natural layout (fp32r, 1.5
                            # cyc/row, ~130ns each); the [128,4,128] f32 tp
                            # tile is byte-identical to a rot_ps slot so it
                            # shares the same 2 PSUM banks
                            tp = pR.tile([128, 4, 128], f32, name="tp", tag="tp")
                            for tb in range(TC8 // 128):
                                nc.tensor.matmul(fr(tp[:, tb, :]), lhsT=fr(vt[:, tb * 128:(tb + 1) * 128]),
                                                 rhs=fr(ident_sb[:]), is_transpose=True,
                                                 skip_group_check=True)
                            nc.vector.tensor_copy(v_sb[:, s0 // 128:s0 // 128 + 4, h * D:(h + 1) * D],
                                                  tp[:, 0:4, :])

                # ============ Phase B: causal attention ============
                with tc.tile_pool(name=f"psB{b}", bufs=1, space="PSUM") as pB:
                    if b + 1 < B:
                        emit_x_dmas(b + 1)
                    for sb in range(NSB):
                        for h in range(HPC):
                            nkb = (sb + 1) * (SB512 // 128)
                            pts = [None] * nkb

                            def emit_score(kblk):
                                delta = kblk - sb * (SB512 // 128)
                                q0 = 128 * delta if delta > 0 else 0
                                W = SB512 - q0
                                q_sl = qT[h][:, sb * SB512 + q0:(sb + 1) * SB512]
                                st_ps = pB.tile([128, SB512], f32, name="st_ps", tag="st", bufs=4)
                                nc.tensor.matmul(st_ps[:, :W], lhsT=kT[h][:, kblk * 128:(kblk + 1) * 128],
                                                 rhs=q_sl, start=True, stop=True)
                                pt = ptp.tile([128, SB512], bf16, name="pt", tag="pt")
                                nc.scalar.activation(pt[:, :W], st_ps[:, :W], Exp, scale=SCALE)
                                if delta >= 0:
                                    nc.gpsimd.affine_select(
                                        out=pt[:, :W], in_=pt[:, :W],
                                        pattern=[[1, W]], compare_op=mybir.AluOpType.is_ge,
                                        fill=0.0, base=0, channel_multiplier=-1,
                                    )
                                pts[kblk] = (pt, q0, W)

                            def emit_zsum(kblk):
                                pt, q0, W = pts[kblk]
                                nc.tensor.matmul(z_ps[:, q0:SB512], lhsT=v_sb[:, kblk, h * D:(h + 1) * D],
                                                 rhs=pt[:, :W], start=(kblk == 0), stop=(kblk == nkb - 1))
                                nc.tensor.matmul(sum_ps[:, q0:SB512], lhsT=ones_sb[:],
                                                 rhs=pt[:, :W], start=(kblk == 0), stop=(kblk == nkb - 1))
                                pts[kblk] = None

                            for kblk in range(min(4, nkb)):
                                emit_score(kblk)
                            z_ps = pB.tile([128, SB512], f32, name="z_ps", tag="z", bufs=2)
                            sum_ps = pB.tile([128, SB512], f32, name="sum_ps", tag="sum", bufs=2)
                            for kblk in range(nkb):
                                if kblk + 4 < nkb:
                                    emit_score(kblk + 4)
                                emit_zsum(kblk)
                            rep_sb = smp.tile([128, SB512], f32, name="rep_sb", tag="repsb")
                            nc.vector.reciprocal_approx_fast(out=rep_sb[:], in_=sum_ps[:])
                            nc.vector.tensor_tensor(zn[h][:, sb * SB512:(sb + 1) * SB512],
                                                    z_ps[:], rep_sb[:], mult)

                # ============ Phase C: output projection ============
                with tc.tile_pool(name=f"psC{b}", bufs=4, space="PSUM") as pC:
                    for tb in range(S // 128):
                        o_sb = op.tile([128, E], bf16, name="o_sb", tag="osb")
                        for ec in range(E // 512):
                            o_ps = pC.tile([128, 512], f32, name="o_ps", tag="o")
                            for h in range(HPC):
                                nc.tensor.matmul(o_ps[:], lhsT=zn[h][:, tb * 128:(tb + 1) * 128],
                                                 rhs=wo_sb[:, h, ec * 512:(ec + 1) * 512],
                                                 start=(h == 0), stop=(h == HPC - 1))
                            if ec % 2 == 0:
                                nc.vector.tensor_copy(o_sb[:, ec * 512:(ec + 1) * 512], o_ps[:])
                            else:
                                nc.scalar.copy(o_sb[:, ec * 512:(ec + 1) * 512], o_ps[:])
                        if b == B - 1 and tb >= S // 128 - 2:
                            # shorten the final drain: last tiles ship in halves
                            nc.sync.dma_start(out=out_d[b * (S // 128) + tb, :, 0:E // 2],
                                              in_=o_sb[:, 0:E // 2])
                            nc.sync.dma_start(out=out_d[b * (S // 128) + tb, :, E // 2:E],
                                              in_=o_sb[:, E // 2:E])
                        else:
                            nc.sync.dma_start(out=out_d[b * (S // 128) + tb], in_=o_sb[:])

    nc.compile()
    return nc


def _get_compiled():
    global _COMPILED
    if _COMPILED is None:
        _COMPILED = _build_program()
    return _COMPILED


def _host_inputs(x, wq, wk, wv, wo):
    bf = ml_dtypes.bfloat16
    x = np.asarray(x, dtype=np.float32)
    # xT blocked: [B*NTC8, 128, KC, TC8]; element (b*NTC8+tc8, p, kc, c) = x[b, tc8*TC8+c, kc*128+p]
    xT = np.ascontiguousarray(
        x.reshape(B, NTC8, TC8, KC, 128).transpose(0, 1, 4, 3, 2).reshape(B * NTC8, 128, KC, TC8)
    ).astype(bf)

    pos = np.arange(S, dtype=np.float32)
    inv_freq = (1.0 / (ROPE_BASE ** (np.arange(0, D, 2, dtype=np.float32) / np.float32(D)))).astype(np.float32)
    ang = pos[:, None] * inv_freq[None, :]          # (S, 64) fp32
    cos_h = np.cos(ang).astype(np.float32)
    sin_h = np.sin(ang).astype(np.float32)
    cosF = np.ascontiguousarray(np.concatenate([cos_h.T, cos_h.T], axis=0)).astype(bf)   # (128, S)
    sinF = np.ascontiguousarray(np.concatenate([-sin_h.T, sin_h.T], axis=0)).astype(bf)  # (128, S)
    ones = np.ones((128, 128), dtype=np.float32).astype(bf)
    ident = np.eye(128, dtype=np.float32)

    wq = np.asarray(wq, dtype=np.float32)
    wk = np.asarray(wk, dtype=np.float32)
    wv = np.asarray(wv, dtype=np.float32)
    wo = np.asarray(wo, dtype=np.float32)

    maps = []
    for c in range(NCORES):
        sl = slice(c * DC, (c + 1) * DC)
        maps.append({
            "xT": xT,
            "cosF": cosF,
            "sinF": sinF,
            "wq": np.ascontiguousarray(wq[:, sl].reshape(KC, 128, DC).transpose(1, 0, 2)).astype(bf),
            "wk": np.ascontiguousarray(wk[:, sl].reshape(KC, 128, DC).transpose(1, 0, 2)).astype(bf),
            "wv": np.ascontiguousarray(wv[:, sl].reshape(KC, 128, DC).transpose(1, 0, 2)).astype(bf),
            "wo": np.ascontiguousarray(wo[sl, :].reshape(HPC, 128, E).transpose(1, 0, 2)).astype(bf),
            "ones": ones,
            "ident": ident,
        })
    return maps


def kernel(x, wq, wk, wv, wo, _trace=False):
    from concourse.bass_utils import run_bass_kernel_spmd

    nc = _get_compiled()
    maps = _host_inputs(x, wq, wk, wv, wo)
    res = run_bass_kernel_spmd(nc, maps, list(range(NCORES)), trace=_trace)
    total = np.zeros((B * (S // 128), 128, E), dtype=np.float32)
    for c in range(NCORES):
        total += res.results[c]["out"].astype(np.float32)
    out = total.reshape(B, S, E)
    if _trace:
        kernel.last_exec_time_ns = res.exec_time_ns
        kernel.last_trace = res.instructions_and_trace
    return out
